# revision 1
# baseline (speedup 1.0000x reference)
"""DeepseekV3 MoE layer on 8 Trainium2 NeuronCores (Bass/Tile).

Sharding:
  - Router: data-parallel (each core routes its own T/8=512 tokens, fp32,
    selection done on exact logits), then AllGather of per-token
    (sel-mask, weight) -> every core knows the full routing.
  - Capacity ranks: per-expert running count over tokens via DVE prefix scan;
    rank <= C survives (matches the reference's stable-sort capacity drop,
    because top-k experts within a token are distinct -> per-expert arrival
    order is token order, and slot order within an expert doesn't affect the
    output).
  - Routed experts: expert-parallel, 4 experts/core.  Token rows are
    dma_gather'ed by compacted slot lists (capacity C=160, padded to 256 per
    expert), GEMM'd, weighted, and dma_scatter_add'ed into a [T, D] partial.
  - Combine: ReduceScatter(add) of partials -> each core owns its 512-token
    slice; adds its locally computed shared-expert MLP and writes the output
    slice.

kernel(**inputs) takes the full unsharded inputs and returns the full
[B, S, D] output.  Self-contained: hardcodes all shapes.
"""

import os
import sys

for _p in ("/opt/trn_rl_repo", "/opt/pypackages"):
    if _p not in sys.path:
        sys.path.insert(0, _p)

import numpy as np

# ---------------------------------------------------------------- constants
B, S, D = 2, 2048, 2048
T = B * S                  # 4096 tokens
I = 1024                   # routed expert intermediate
E = 32                     # routed experts
K = 4                      # experts per token
NG = 8                     # groups
GS = E // NG               # experts per group = 4
TKG = 3                    # top-k groups
ISH = 2048                 # shared expert intermediate (I * n_shared)
SCALE = 2.5
C = 160                    # capacity = ceil(1.25 * T / E)
CP = 256                   # per-expert slot padding (128-aligned)
NCORES = 8
EL = E // NCORES           # local experts per core = 4
TL = T // NCORES           # local tokens per core = 512
NSLOT = EL * CP            # padded slots per core = 1024

# "f32" | "f32r" | "bf16" : dtype/mode of the heavy GEMMs (router stays f32)
GEMM_MODE = os.environ.get("BASS_MOE_GEMM_MODE", "f32")


# ---------------------------------------------------------------- builder
def _build(gemm_mode: str):
    import concourse.bass as bass
    import concourse.bacc as bacc
    import concourse.mybir as mybir
    import concourse.tile as tile
    from concourse import masks
    from contextlib import ExitStack

    dt = mybir.dt
    Alu = mybir.AluOpType
    Act = mybir.ActivationFunctionType

    f32 = dt.float32
    bf16 = dt.bfloat16
    wdt = bf16 if gemm_mode == "bf16" else f32

    def mm_cast(ap):
        if gemm_mode == "f32r":
            return ap.bitcast(dt.float32r)
        return ap

    nc = bacc.Bacc(None, num_devices=NCORES, num_swdge_queues=1)
    groups = [list(range(NCORES))]

    # ---------------- I/O ----------------
    x_full = nc.dram_tensor("x_full", [T, D], wdt, kind="ExternalInput")
    x_own = nc.dram_tensor("x_own", [TL, D], f32, kind="ExternalInput")
    rwT = nc.dram_tensor("rwT", [D, E], f32, kind="ExternalInput")
    ebias = nc.dram_tensor("ebias", [1, E], f32, kind="ExternalInput")
    sloc = nc.dram_tensor("sloc", [2 * E, 36], f32, kind="ExternalInput")
    wg = nc.dram_tensor("wg", [EL, D, I], wdt, kind="ExternalInput")
    wu = nc.dram_tensor("wu", [EL, D, I], wdt, kind="ExternalInput")
    wd = nc.dram_tensor("wd", [EL, I, D], wdt, kind="ExternalInput")
    sgT = nc.dram_tensor("sgT", [D, ISH], wdt, kind="ExternalInput")
    suT = nc.dram_tensor("suT", [D, ISH], wdt, kind="ExternalInput")
    sdT = nc.dram_tensor("sdT", [ISH, D], wdt, kind="ExternalInput")
    out = nc.dram_tensor("out", [TL, D], f32, kind="ExternalOutput")

    # ---------------- internal DRAM ----------------
    selw_own = nc.dram_tensor("selw_own", [TL, 2 * E], f32)
    selw_all = nc.dram_tensor("selw_all", [T, 2 * E], f32, addr_space="Shared")
    partial = nc.dram_tensor("partial", [T, D], f32)
    rs_out = nc.dram_tensor("rs_out", [TL, D], f32)
    shr_out = nc.dram_tensor("shr_out", [TL, D], f32)
    idx_dram = nc.dram_tensor("idx_dram", [16, EL * 16], dt.int16)
    at_dram = nc.dram_tensor("at_dram", [EL, T], f32)
    nf_dram = nc.dram_tensor("nf_dram", [1, EL], f32)
    aw_dram = nc.dram_tensor("aw_dram", [EL, T], f32)

    DC = D // 128            # 16 d-chunks
    IC = I // 128            # 8  i-chunks
    MC = ISH // 128          # 16 shared-intermediate chunks
    TT = TL // 128           # 4 own-token tiles
    NT = T // 128            # 32 all-token tiles
    CH = 4                   # routing-table token chunks
    CT = T // CH             # 1024 tokens per chunk

    with tile.TileContext(nc) as tc, ExitStack() as ctx:
        consts = ctx.enter_context(tc.tile_pool(name="consts", bufs=1))
        work = ctx.enter_context(tc.tile_pool(name="work", bufs=2))
        psum_t = ctx.enter_context(
            tc.tile_pool(name="psum_t", bufs=2, space="PSUM"))
        psum_g = ctx.enter_context(
            tc.tile_pool(name="psum_g", bufs=2, space="PSUM"))
        psum_u = ctx.enter_context(
            tc.tile_pool(name="psum_u", bufs=2, space="PSUM"))
        psum_y = ctx.enter_context(
            tc.tile_pool(name="psum_y", bufs=2, space="PSUM"))
        persist = ctx.enter_context(tc.tile_pool(name="persist", bufs=1))
        wstream = ctx.enter_context(tc.tile_pool(name="wstream", bufs=2))

        # ---------------- constants ----------------
        ident = consts.tile([128, 128], f32)
        masks.make_identity(nc, ident[:])
        if wdt != f32:
            ident_w = consts.tile([128, 128], wdt)
            nc.vector.tensor_copy(ident_w[:], ident[:])
        else:
            ident_w = ident

        ebias_b = consts.tile([128, E], f32)
        nc.sync.dma_start(ebias_b[:], ebias[0:1, :].broadcast_to([128, E]))

        negbuf = consts.tile([128, E], f32)
        nc.gpsimd.memset(negbuf[:], -1e30)

        iota16_i = consts.tile([16, 16], dt.int32)
        nc.gpsimd.iota(iota16_i[:], pattern=[[16, 16]], base=0,
                       channel_multiplier=1)
        iota16 = consts.tile([16, 16], f32)
        nc.vector.tensor_copy(iota16[:], iota16_i[:])

        # zero-fill the [T, D] partial early (overlaps with compute)
        zt = consts.tile([128, 512], f32)
        nc.gpsimd.memset(zt[:], 0.0)
        for r in range(NT):
            for zc in range(D // 512):
                nc.sync.dma_start(
                    partial[r * 128:(r + 1) * 128,
                            zc * 512:(zc + 1) * 512], zt[:])

        # ---------------- P1: transpose own tokens -> xT [128, DC, TL] ----
        xtp_cm = tc.tile_pool(name="xtp", bufs=1)
        xtp = xtp_cm.__enter__()
        xT = xtp.tile([128, DC, TL], f32)
        for tt in range(TT):
            for dc2 in range(DC // 2):
                xtile = work.tile([128, 256], f32, tag="xtile")
                nc.sync.dma_start(
                    xtile[:],
                    x_own[tt * 128:(tt + 1) * 128, dc2 * 256:(dc2 + 1) * 256])
                for h in range(2):
                    dc = dc2 * 2 + h
                    pt = psum_t.tile([128, 128], f32, tag="pt")
                    nc.tensor.transpose(
                        pt[:], xtile[:, h * 128:(h + 1) * 128], ident[:])
                    nc.vector.tensor_copy(
                        xT[:, dc, tt * 128:(tt + 1) * 128], pt[:])
        if wdt != f32:
            xTw = xtp.tile([128, DC, TL], wdt)
            for dc in range(DC):
                nc.vector.tensor_copy(xTw[:, dc, :], xT[:, dc, :])
        else:
            xTw = xT

        # ---------------- P2: router on own tokens (fp32/exact) -----------
        rwT_sb = consts.tile([128, DC, E], f32)
        nc.sync.dma_start(
            rwT_sb[:], rwT[:].rearrange("(c p) e -> p c e", p=128))

        for tt in range(TT):
            ps = psum_t.tile([128, E], f32, tag="pt")
            for dc in range(DC):
                nc.tensor.matmul(
                    ps[:], xT[:, dc, tt * 128:(tt + 1) * 128], rwT_sb[:, dc, :],
                    start=(dc == 0), stop=(dc == DC - 1))
            L = work.tile([128, E], f32, tag="rL")
            nc.vector.tensor_copy(L[:], ps[:])
            Ssig = work.tile([128, E], f32, tag="rS")
            nc.scalar.activation(Ssig[:], ps[:], Act.Sigmoid)
            Sb = work.tile([128, E], f32, tag="rSb")
            nc.vector.tensor_tensor(Sb[:], Ssig[:], ebias_b[:], op=Alu.add)

            # group score = top-2 sum per group = max over pair sums
            Sv = Sb[:].rearrange("p (g i) -> p g i", i=GS)
            gs = work.tile([128, NG], f32, tag="rGS")
            tmp = work.tile([128, NG], f32, tag="rtmp")
            nc.vector.tensor_tensor(gs[:], Sv[:, :, 0], Sv[:, :, 1], op=Alu.add)
            for (a, b) in [(0, 2), (0, 3), (1, 2), (1, 3), (2, 3)]:
                nc.vector.tensor_tensor(
                    tmp[:], Sv[:, :, a], Sv[:, :, b], op=Alu.add)
                nc.vector.tensor_tensor(gs[:], gs[:], tmp[:], op=Alu.max)

            m8g = work.tile([128, 8], f32, tag="rm8g")
            nc.vector.max(m8g[:], gs[:])
            gmask = work.tile([128, NG], f32, tag="rgm")
            nc.vector.tensor_scalar(
                gmask[:], gs[:], m8g[:, TKG - 1:TKG], None, op0=Alu.is_ge)

            emask = work.tile([128, E], f32, tag="rem")
            emv = emask[:].rearrange("p (g i) -> p g i", i=GS)
            for r in range(GS):
                nc.vector.tensor_copy(emv[:, :, r], gmask[:])

            # top-4 experts among unmasked, compared on exact logits
            emask8 = work.tile([128, E], dt.uint8, tag="rem8")
            nc.vector.tensor_copy(emask8[:], emask[:])
            ml = work.tile([128, E], f32, tag="rml")
            nc.vector.tensor_copy(ml[:], negbuf[:])
            nc.vector.copy_predicated(ml[:], emask8[:], L[:])
            m8e = work.tile([128, 8], f32, tag="rm8e")
            nc.vector.max(m8e[:], ml[:])
            sel = work.tile([128, E], f32, tag="rsel")
            nc.vector.tensor_scalar(
                sel[:], ml[:], m8e[:, K - 1:K], None, op0=Alu.is_ge)

            wm = work.tile([128, E], f32, tag="rwm")
            nc.vector.tensor_tensor(wm[:], Ssig[:], sel[:], op=Alu.mult)
            den = work.tile([128, 1], f32, tag="rden")
            nc.vector.tensor_reduce(
                den[:], wm[:], axis=mybir.AxisListType.X, op=Alu.add)
            nc.vector.tensor_scalar(den[:], den[:], 1e-20, None, op0=Alu.add)
            winv = work.tile([128, 1], f32, tag="rwinv")
            nc.vector.reciprocal(winv[:], den[:])

            sw = work.tile([128, 2 * E], f32, tag="rsw")
            nc.vector.tensor_copy(sw[:, 0:E], sel[:])
            nc.vector.tensor_scalar(
                sw[:, E:2 * E], wm[:], winv[:, 0:1], SCALE,
                op0=Alu.mult, op1=Alu.mult)
            nc.sync.dma_start(selw_own[tt * 128:(tt + 1) * 128, :], sw[:])

        # ---------------- P8a: shared expert gate/up (independent) --------
        HsT = persist.tile([128, MC, TL], wdt)
        for mc in range(MC):
            sg_t = wstream.tile([128, DC, 128], wdt, tag="wst")
            nc.sync.dma_start(
                sg_t[:],
                sgT[:].rearrange("(c p) i -> p c i", p=128)
                [:, :, mc * 128:(mc + 1) * 128])
            su_t = wstream.tile([128, DC, 128], wdt, tag="wst2")
            nc.sync.dma_start(
                su_t[:],
                suT[:].rearrange("(c p) i -> p c i", p=128)
                [:, :, mc * 128:(mc + 1) * 128])
            pg = psum_g.tile([128, TL], f32, tag="pg")
            pu = psum_u.tile([128, TL], f32, tag="pu")
            for dc in range(DC):
                nc.tensor.matmul(
                    pg[:], mm_cast(sg_t[:, dc, :]), mm_cast(xTw[:, dc, :]),
                    start=(dc == 0), stop=(dc == DC - 1))
            for dc in range(DC):
                nc.tensor.matmul(
                    pu[:], mm_cast(su_t[:, dc, :]), mm_cast(xTw[:, dc, :]),
                    start=(dc == 0), stop=(dc == DC - 1))
            sig = work.tile([128, TL], f32, tag="ssig")
            nc.scalar.activation(sig[:], pg[:], Act.Sigmoid)
            sil = work.tile([128, TL], wdt, tag="ssil")
            nc.vector.tensor_tensor(sil[:], sig[:], pg[:], op=Alu.mult)
            nc.vector.tensor_tensor(HsT[:, mc, :], sil[:], pu[:], op=Alu.mult)

        xtp_cm.__exit__(None, None, None)

        # ---------------- P3: AllGather routing ----------------
        nc.gpsimd.collective_compute(
            "AllGather", Alu.bypass, replica_groups=groups,
            ins=[selw_own[:]], outs=[selw_all[:]])

        # ---------------- P4: routing tables (chunked over tokens) --------
        sloc_sb = consts.tile([64, 36], f32)
        nc.sync.dma_start(sloc_sb[:], sloc[:])

        sgin_t = persist.tile([16, EL, T // 16], f32)
        sgin_w = persist.tile([16, EL, T // 16], f32)
        carry = persist.tile([EL, 1], f32)
        nc.gpsimd.memset(carry[:], 0.0)

        route_cm = tc.tile_pool(name="route", bufs=1)
        route = route_cm.__enter__()
        for q in range(CH):
            selwT_c = route.tile([64, CT // 128, 128], f32, tag="selwT")
            for j in range(CT // 128):
                tt = q * (CT // 128) + j
                swt = work.tile([128, 2 * E], f32, tag="swt")
                nc.sync.dma_start(
                    swt[:], selw_all[tt * 128:(tt + 1) * 128, :])
                pt = psum_t.tile([64, 128], f32, tag="pt")
                nc.tensor.transpose(pt[:], swt[:], ident[:])
                nc.vector.tensor_copy(selwT_c[:, j, :], pt[:])

            SW_sel = route.tile([EL, CT], f32, tag="SWsel")
            SW_w = route.tile([EL, CT], f32, tag="SWw")
            for h in range(CT // 512):
                pswl = psum_g.tile([36, 512], f32, tag="pg")
                nc.tensor.matmul(
                    pswl[:], sloc_sb[:], selwT_c[:, 4 * h:4 * (h + 1), :],
                    start=True, stop=True)
                nc.vector.tensor_copy(
                    SW_sel[:, h * 512:(h + 1) * 512], pswl[0:EL, :])
                nc.vector.tensor_copy(
                    SW_w[:, h * 512:(h + 1) * 512], pswl[32:36, :])

            rank_c = route.tile([EL, CT], f32, tag="rankc")
            nc.vector.tensor_tensor_scan(
                rank_c[:], SW_sel[:], SW_sel[:], carry[:, 0:1],
                op0=Alu.add, op1=Alu.bypass)
            nc.vector.tensor_copy(carry[:], rank_c[:, CT - 1:CT])

            fsel_c = route.tile([EL, CT], f32, tag="fselc")
            nc.vector.tensor_scalar(
                fsel_c[:], rank_c[:], float(C), None, op0=Alu.is_le)
            nc.vector.tensor_tensor(
                fsel_c[:], fsel_c[:], SW_sel[:], op=Alu.mult)

            iota_i = route.tile([EL, CT], dt.int32, tag="iotai")
            nc.gpsimd.iota(iota_i[:], pattern=[[1, CT]], base=1 + q * CT,
                           channel_multiplier=0)
            iota_f = route.tile([EL, CT], f32, tag="iotaf")
            nc.vector.tensor_copy(iota_f[:], iota_i[:])

            At_c = route.tile([EL, CT], f32, tag="Atc")
            nc.vector.tensor_tensor(At_c[:], fsel_c[:], iota_f[:], op=Alu.mult)
            nc.vector.tensor_scalar(At_c[:], At_c[:], 1.0, None,
                                    op0=Alu.subtract)

            fsel8 = route.tile([EL, CT], dt.uint8, tag="fsel8")
            nc.vector.tensor_copy(fsel8[:], fsel_c[:])
            Aw_c = route.tile([EL, CT], f32, tag="Awc")
            nc.gpsimd.memset(Aw_c[:], -1.0)
            nc.vector.copy_predicated(Aw_c[:], fsel8[:], SW_w[:])

            nc.sync.dma_start(at_dram[:, q * CT:(q + 1) * CT], At_c[:])
            nc.sync.dma_start(aw_dram[:, q * CT:(q + 1) * CT], Aw_c[:])

        for e in range(EL):
            nc.sync.dma_start(
                sgin_t[:, e, :],
                at_dram[e].rearrange("(c b) -> b c", b=16))
            nc.sync.dma_start(
                sgin_w[:, e, :],
                aw_dram[e].rearrange("(c b) -> b c", b=16))

        # per-expert compaction -> slot lists + weights
        idx16s = persist.tile([16, EL * 16], dt.int16)   # 16-row wrapped
        idx16 = persist.tile([128, EL * 16], dt.int16)   # replicated to 128
        w_col = persist.tile([128, 2 * EL], f32)

        sgtoks, sgws = [], []
        for e in range(EL):
            sgtok = work.tile([16, 16], f32, tag=f"sgtok{e}")
            nft = work.tile([1, 1], dt.uint32, tag=f"nft{e}")
            nc.gpsimd.sparse_gather(sgtok[:], sgin_t[:, e, :], num_found=nft[:])
            sgw = work.tile([16, 16], f32, tag=f"sgw{e}")
            nfw = work.tile([1, 1], dt.uint32, tag=f"nfw{e}")
            nc.gpsimd.sparse_gather(sgw[:], sgin_w[:, e, :], num_found=nfw[:])
            nf_f = work.tile([1, 1], f32, tag=f"nf_f{e}")
            nc.vector.tensor_copy(nf_f[:], nft[:])
            nc.sync.dma_start(nf_dram[0:1, e:e + 1], nf_f[:])
            sgtoks.append(sgtok)
            sgws.append(sgw)

        for e in range(EL):
            sgtok, sgw = sgtoks[e], sgws[e]
            nf16 = work.tile([16, 1], f32, tag=f"nf16{e}")
            nc.sync.dma_start(
                nf16[:], nf_dram[0:1, e:e + 1].broadcast_to([16, 1]))
            vm = work.tile([16, 16], f32, tag=f"vm{e}")
            nc.vector.tensor_scalar(
                vm[:], iota16[:], nf16[:, 0:1], None, op0=Alu.is_lt)

            tokm = work.tile([16, 16], f32, tag=f"tokm{e}")
            nc.vector.tensor_tensor(tokm[:], sgtok[:], vm[:], op=Alu.mult)
            nc.vector.tensor_copy(idx16s[:, 16 * e:16 * (e + 1)], tokm[:])

            wsl = work.tile([16, 16], f32, tag=f"wsl{e}")
            nc.vector.tensor_tensor(wsl[:], sgw[:], vm[:], op=Alu.mult)
            ptw = psum_t.tile([16, 16], f32, tag="pt")
            nc.tensor.transpose(ptw[:], wsl[:], ident[:16, :16])
            wt16 = work.tile([16, 16], f32, tag=f"wt16{e}")
            nc.vector.tensor_copy(wt16[:], ptw[:])
            nc.sync.dma_start(w_col[:, 2 * e:2 * e + 1], wt16[0:8, :])
            nc.sync.dma_start(w_col[:, 2 * e + 1:2 * e + 2], wt16[8:16, :])

        # replicate the 16-row wrapped index block to all 128 partitions
        nc.sync.dma_start(idx_dram[:], idx16s[:])
        for r in range(8):
            nc.sync.dma_start(idx16[16 * r:16 * (r + 1), :], idx_dram[:])

        route_cm.__exit__(None, None, None)

        # ---------------- P5..P7: dispatch + routed expert GEMMs ----------
        with tc.tile_pool(name="dpXP", bufs=2) as dpXP, \
                tc.tile_pool(name="dpXPT", bufs=1) as dpXPT, \
                tc.tile_pool(name="dpHT", bufs=1) as dpHT, \
                tc.tile_pool(name="dpWD", bufs=1) as dpWD:
            for e in range(EL):
                XP = dpXP.tile([128, CP // 128, D], wdt, tag="XP")
                nc.gpsimd.dma_gather(
                    XP[:], x_full[:], idx16[:, 16 * e:16 * (e + 1)],
                    CP, CP, D, queue_num=0)
                XPT = dpXPT.tile([128, DC, CP], wdt, tag="XPT")
                for col in range(CP // 128):
                    for dc in range(DC):
                        ptx = psum_t.tile([128, 128], wdt, tag="pt")
                        nc.tensor.transpose(
                            ptx[:], XP[:, col, dc * 128:(dc + 1) * 128],
                            ident_w[:])
                        nc.vector.tensor_copy(
                            XPT[:, dc, col * 128:(col + 1) * 128], ptx[:])

                HT = dpHT.tile([128, IC, CP], wdt, tag="HT")
                for ic in range(IC):
                    wg_t = wstream.tile([128, DC, 128], wdt, tag="wst")
                    nc.sync.dma_start(
                        wg_t[:],
                        wg[e].rearrange("(c p) i -> p c i", p=128)
                        [:, :, ic * 128:(ic + 1) * 128])
                    wu_t = wstream.tile([128, DC, 128], wdt, tag="wst2")
                    nc.sync.dma_start(
                        wu_t[:],
                        wu[e].rearrange("(c p) i -> p c i", p=128)
                        [:, :, ic * 128:(ic + 1) * 128])
                    pg = psum_g.tile([128, CP], f32, tag="pg")
                    pu = psum_u.tile([128, CP], f32, tag="pu")
                    for dc in range(DC):
                        nc.tensor.matmul(
                            pg[:], mm_cast(wg_t[:, dc, :]),
                            mm_cast(XPT[:, dc, :]),
                            start=(dc == 0), stop=(dc == DC - 1))
                    for dc in range(DC):
                        nc.tensor.matmul(
                            pu[:], mm_cast(wu_t[:, dc, :]),
                            mm_cast(XPT[:, dc, :]),
                            start=(dc == 0), stop=(dc == DC - 1))
                    sig = work.tile([128, CP], f32, tag="esig")
                    nc.scalar.activation(sig[:], pg[:], Act.Sigmoid)
                    sil = work.tile([128, CP], wdt, tag="esil")
                    nc.vector.tensor_tensor(sil[:], sig[:], pg[:], op=Alu.mult)
                    nc.vector.tensor_tensor(
                        HT[:, ic, :], sil[:], pu[:], op=Alu.mult)

                for dc4 in range(D // 512):
                    wd_t = dpWD.tile([128, IC, 512], wdt, tag="wst3")
                    nc.sync.dma_start(
                        wd_t[:],
                        wd[e].rearrange("(c p) d -> p c d", p=128)
                        [:, :, dc4 * 512:(dc4 + 1) * 512])
                    for sb in range(2):
                        py = psum_y.tile([128, 512], f32, tag="py")
                        for ic in range(IC):
                            nc.tensor.matmul(
                                py[:],
                                mm_cast(HT[:, ic, sb * 128:(sb + 1) * 128]),
                                mm_cast(wd_t[:, ic, :]),
                                start=(ic == 0), stop=(ic == IC - 1))
                        yw = work.tile([128, 512], f32, tag="yw")
                        nc.vector.tensor_scalar(
                            yw[:], py[:],
                            w_col[:, 2 * e + sb:2 * e + sb + 1], None,
                            op0=Alu.mult)
                        nc.gpsimd.dma_scatter_add(
                            partial[:, dc4 * 512:(dc4 + 1) * 512],
                            yw[:].rearrange("p (a f) -> p a f", a=1),
                            idx16[:, 16 * e + 8 * sb:16 * e + 8 * sb + 8],
                            128, 128, 512, elem_step=D, queue_num=0)

        # ---------------- P8b: shared expert down proj -> shr_out ---------
        with tc.tile_pool(name="sdpool", bufs=1) as sdpool:
            for dc4 in range(D // 512):
                sd_t = sdpool.tile([128, MC, 512], wdt, tag="wsd")
                nc.sync.dma_start(
                    sd_t[:],
                    sdT[:].rearrange("(c p) d -> p c d", p=128)
                    [:, :, dc4 * 512:(dc4 + 1) * 512])
                for tb in range(TT):
                    po = psum_y.tile([128, 512], f32, tag="py")
                    for mc in range(MC):
                        nc.tensor.matmul(
                            po[:], mm_cast(HsT[:, mc, tb * 128:(tb + 1) * 128]),
                            mm_cast(sd_t[:, mc, :]),
                            start=(mc == 0), stop=(mc == MC - 1))
                    ot = work.tile([128, 512], f32, tag="ot")
                    nc.vector.tensor_copy(ot[:], po[:])
                    nc.sync.dma_start(
                        shr_out[tb * 128:(tb + 1) * 128,
                                dc4 * 512:(dc4 + 1) * 512], ot[:])

        # ---------------- P9: ReduceScatter + final add ----------------
        nc.gpsimd.collective_compute(
            "ReduceScatter", Alu.add, replica_groups=groups,
            ins=[partial[:]], outs=[rs_out[:]])

        for tb in range(TT):
            for dc4 in range(D // 512):
                rst = work.tile([128, 512], f32, tag="rst")
                nc.sync.dma_start(
                    rst[:], rs_out[tb * 128:(tb + 1) * 128,
                                   dc4 * 512:(dc4 + 1) * 512])
                sht = work.tile([128, 512], f32, tag="sht")
                nc.sync.dma_start(
                    sht[:], shr_out[tb * 128:(tb + 1) * 128,
                                    dc4 * 512:(dc4 + 1) * 512])
                fin = work.tile([128, 512], f32, tag="fin")
                nc.vector.tensor_tensor(fin[:], sht[:], rst[:], op=Alu.add)
                nc.sync.dma_start(
                    out[tb * 128:(tb + 1) * 128,
                        dc4 * 512:(dc4 + 1) * 512], fin[:])

    nc.finalize()
    return nc


_NC_CACHE = {}


def get_nc(gemm_mode=None):
    gemm_mode = gemm_mode or GEMM_MODE
    if gemm_mode not in _NC_CACHE:
        _NC_CACHE[gemm_mode] = _build(gemm_mode)
    return _NC_CACHE[gemm_mode]


def make_in_maps(inputs, gemm_mode=None):
    """Shard full inputs into the 8 per-core input maps."""
    import ml_dtypes

    gemm_mode = gemm_mode or GEMM_MODE
    wnp = ml_dtypes.bfloat16 if gemm_mode == "bf16" else np.float32

    x = np.asarray(inputs["hidden_states"], np.float32).reshape(T, D)
    router_w = np.asarray(inputs["router_w"], np.float32)
    e_bias = np.asarray(inputs["e_bias"], np.float32).reshape(1, E)
    W_gate = np.asarray(inputs["W_gate"])
    W_up = np.asarray(inputs["W_up"])
    W_down = np.asarray(inputs["W_down"])
    shared_gate = np.asarray(inputs["shared_gate"], np.float32)
    shared_up = np.asarray(inputs["shared_up"], np.float32)
    shared_down = np.asarray(inputs["shared_down"], np.float32)

    x_w = np.ascontiguousarray(x).astype(wnp)
    rwT = np.ascontiguousarray(router_w.T.astype(np.float32))
    sgT = np.ascontiguousarray(shared_gate.T).astype(wnp)
    suT = np.ascontiguousarray(shared_up.T).astype(wnp)
    sdT = np.ascontiguousarray(shared_down.T).astype(wnp)

    in_maps = []
    for c in range(NCORES):
        sl = np.zeros((2 * E, 36), np.float32)
        for j in range(EL):
            sl[EL * c + j, j] = 1.0
            sl[E + EL * c + j, 32 + j] = 1.0
        in_maps.append({
            "x_full": x_w,
            "x_own": np.ascontiguousarray(x[TL * c:TL * (c + 1)]),
            "rwT": rwT,
            "ebias": e_bias,
            "sloc": sl,
            "wg": np.ascontiguousarray(W_gate[EL * c:EL * (c + 1)]).astype(wnp),
            "wu": np.ascontiguousarray(W_up[EL * c:EL * (c + 1)]).astype(wnp),
            "wd": np.ascontiguousarray(W_down[EL * c:EL * (c + 1)]).astype(wnp),
            "sgT": sgT,
            "suT": suT,
            "sdT": sdT,
        })
    return in_maps


def kernel(**inputs):
    from concourse.bass_utils import run_bass_kernel_spmd

    nc = get_nc()
    in_maps = make_in_maps(inputs)
    trace = bool(int(os.environ.get("BASS_MOE_TRACE", "0")))
    res = run_bass_kernel_spmd(
        nc, in_maps, core_ids=list(range(NCORES)), trace=trace)
    if trace and res.exec_time_ns is not None:
        print(f"HW exec time: {res.exec_time_ns} ns")
        kernel.last_exec_time_ns = res.exec_time_ns
    out = np.concatenate([res.results[c]["out"] for c in range(NCORES)], axis=0)
    return out.reshape(B, S, D)


kernel.last_exec_time_ns = None



# revision 7
# speedup vs baseline: 4.3591x; 4.3591x over previous
"""DeepseekV3 MoE layer on 8 Trainium2 NeuronCores (Bass/Tile), v2.

Sharding (same topology as v1):
  - Router: data-parallel (each core routes its own T/8=512 tokens, fp32,
    selection done on exact logits), then AllGather of per-token
    (sel-mask, weight) -> every core knows the full routing.
  - Routed experts: expert-parallel, 4 experts/core; dma_gather rows,
    GEMM, dma_scatter_add into a [T, D] partial.
  - Combine: ReduceScatter(add) + locally computed shared-expert MLP.

v2 changes vs v1:
  - Heavy GEMMs in bf16 (or f32r): 4x PE throughput; router stays fp32.
  - All weights pre-laid out on host into ONE contiguous [128, COLS]
    blob in exact kernel read order -> every weight DMA is a wide
    contiguous column slice (>=4KB per partition line).
  - x transposed on host -> P1 on-device transpose removed.
  - Inputs packed: 4 tensors/core instead of 11 (less per-exec staging).
  - ReduceScatter issued before the shared-expert down-projection so the
    collective overlaps PE work.

kernel(**inputs) takes the full unsharded inputs and returns the full
[B, S, D] output.  Self-contained: hardcodes all shapes.
"""

import os
import sys

for _p in ("/opt/trn_rl_repo", "/opt/pypackages"):
    if _p not in sys.path:
        sys.path.insert(0, _p)

import numpy as np

# ---------------------------------------------------------------- constants
B, S, D = 2, 2048, 2048
T = B * S                  # 4096 tokens
I = 1024                   # routed expert intermediate
E = 32                     # routed experts
K = 4                      # experts per token
NG = 8                     # groups
GS = E // NG               # experts per group = 4
TKG = 3                    # top-k groups
ISH = 2048                 # shared expert intermediate (I * n_shared)
SCALE = 2.5
C = 160                    # capacity = ceil(1.25 * T / E)
CP = 256                   # per-expert slot padding (128-aligned)
CPV = 192                  # processed slots per expert (>= C, 64-multiple)
NCORES = 8
EL = E // NCORES           # local experts per core = 4
TL = T // NCORES           # local tokens per core = 512
NSLOT = EL * CP            # padded slots per core = 1024

DC = D // 128              # 16 d-chunks
IC = I // 128              # 8  i-chunks
MC = ISH // 128            # 16 shared-intermediate chunks
TT = TL // 128             # 4 own-token tiles
NT = T // 128              # 32 all-token tiles
CH = 4                     # routing-table token chunks
CT = T // CH               # 1024 tokens per chunk

# weight blob column offsets (in elements of wdt), [128, WCOLS]
# order: shared gate/up (per mc), shared down (per dc4),
#        per expert: gate/up (per ic), down (per dc4)
SG_COLS = DC * 128         # 2048 per mc (gate), same for up
SD_COLS = MC * 512         # 8192 per dc4
WG_COLS = DC * 128         # 2048 per (e, ic)
WD_COLS = IC * 512         # 4096 per (e, dc4)
OFF_SHGU = 0                                   # mc-major: [gate, up] pairs
OFF_SHD = OFF_SHGU + MC * 2 * SG_COLS          # 65536
OFF_EXP = OFF_SHD + 4 * SD_COLS                # 98304
EXP_STRIDE = IC * 2 * WG_COLS + 4 * WD_COLS    # 49152 per expert
WCOLS = OFF_EXP + EL * EXP_STRIDE              # 294912

# f32 small blob: [128, SCOLS]: rwT (DC*E) | ebias (E) | sloc (36)
RW_COLS = DC * E           # 512
OFF_EB = RW_COLS
OFF_SL = RW_COLS + E
SCOLS = RW_COLS + E + 36   # 580

# "bf16" | "f32r" : dtype of the heavy GEMMs (router stays f32)
GEMM_MODE = os.environ.get("BASS_MOE_GEMM_MODE", "bf16")
# dtype of the partial-sum buffer + ReduceScatter ("bf16" | "f32")
PART_DT = os.environ.get("BASS_MOE_PART_DT", "bf16")


# ---------------------------------------------------------------- builder
def _build(gemm_mode: str):
    import concourse.bass as bass
    import concourse.bacc as bacc
    import concourse.mybir as mybir
    import concourse.tile as tile
    from concourse import masks
    from contextlib import ExitStack

    dt = mybir.dt
    Alu = mybir.AluOpType
    Act = mybir.ActivationFunctionType

    f32 = dt.float32
    wdt = dt.bfloat16 if gemm_mode == "bf16" else dt.float32r
    pdt = dt.bfloat16 if PART_DT == "bf16" else f32

    nc = bacc.Bacc(None, num_devices=NCORES, num_swdge_queues=1)
    groups = [list(range(NCORES))]

    # ---------------- I/O ----------------
    x_full = nc.dram_tensor("x_full", [T, D], wdt, kind="ExternalInput")
    xT_in = nc.dram_tensor("xT_in", [128, DC * TL], f32, kind="ExternalInput")
    wcat = nc.dram_tensor("wcat", [128, WCOLS], wdt, kind="ExternalInput")
    scat = nc.dram_tensor("scat", [128, SCOLS], f32, kind="ExternalInput")
    out = nc.dram_tensor("out", [TL, D], f32, kind="ExternalOutput")

    # ---------------- internal DRAM ----------------
    selw_own = nc.dram_tensor("selw_own", [TL, 2 * E], f32)
    selw_all = nc.dram_tensor("selw_all", [T, 2 * E], f32, addr_space="Shared")
    partials = [nc.dram_tensor(f"partial{j}", [T, 512], pdt)
                for j in range(D // 512)]
    rs_outs = [nc.dram_tensor(f"rs_out{j}", [TL, 512], pdt)
               for j in range(D // 512)]
    idx_dram = nc.dram_tensor("idx_dram", [16, EL * 16], dt.int16)
    at_dram = nc.dram_tensor("at_dram", [EL, T], f32)
    nf_dram = nc.dram_tensor("nf_dram", [1, EL], f32)
    aw_dram = nc.dram_tensor("aw_dram", [EL, T], f32)

    with tile.TileContext(nc) as tc, ExitStack() as ctx:
        consts = ctx.enter_context(tc.tile_pool(name="consts", bufs=1))
        work = ctx.enter_context(tc.tile_pool(name="work", bufs=2))
        psum_t = ctx.enter_context(
            tc.tile_pool(name="psum_t", bufs=2, space="PSUM"))
        psum_g = ctx.enter_context(
            tc.tile_pool(name="psum_g", bufs=2, space="PSUM"))
        psum_u = ctx.enter_context(
            tc.tile_pool(name="psum_u", bufs=2, space="PSUM"))
        psum_y = ctx.enter_context(
            tc.tile_pool(name="psum_y", bufs=2, space="PSUM"))
        persist = ctx.enter_context(tc.tile_pool(name="persist", bufs=1))
        wstream = ctx.enter_context(tc.tile_pool(name="wstream", bufs=2))

        # ---------------- inputs -> SBUF ----------------
        scat_sb = consts.tile([128, SCOLS], f32)
        nc.sync.dma_start(scat_sb[:], scat[:])
        rwT_sb = scat_sb[:, 0:RW_COLS].rearrange("p (c e) -> p c e", e=E)
        ebias_b = scat_sb[:, OFF_EB:OFF_EB + E]
        sloc_sb = scat_sb[0:64, OFF_SL:OFF_SL + 36]

        xtp_cm = tc.tile_pool(name="xtp", bufs=1)
        xtp = xtp_cm.__enter__()
        xT = xtp.tile([128, DC, TL], f32)
        nc.sync.dma_start(
            xT[:], xT_in[:].rearrange("p (c t) -> p c t", t=TL))

        # ---------------- constants ----------------
        ident = consts.tile([128, 128], f32)
        masks.make_identity(nc, ident[:])
        ident_w = consts.tile([128, 128], wdt)
        nc.vector.tensor_copy(ident_w[:], ident[:])

        negbuf = consts.tile([128, E], f32)
        nc.gpsimd.memset(negbuf[:], -1e30)

        iota16_i = consts.tile([16, 16], dt.int32)
        nc.gpsimd.iota(iota16_i[:], pattern=[[16, 16]], base=0,
                       channel_multiplier=1)
        iota16 = consts.tile([16, 16], f32)
        nc.vector.tensor_copy(iota16[:], iota16_i[:])

        # zero-fill the [T, D] partial (overlaps with router/shared gemm)
        zt = consts.tile([128, 512], pdt)
        nc.gpsimd.memset(zt[:], 0.0)
        for r in range(NT):
            for zc in range(D // 512):
                nc.sync.dma_start(
                    partials[zc][r * 128:(r + 1) * 128, :], zt[:])

        # ---------------- P2: router on own tokens (fp32/exact) -----------
        for tt in range(TT):
            ps = psum_t.tile([128, E], f32, tag="pt")
            for dc in range(DC):
                nc.tensor.matmul(
                    ps[:], xT[:, dc, tt * 128:(tt + 1) * 128], rwT_sb[:, dc, :],
                    start=(dc == 0), stop=(dc == DC - 1))
            L = work.tile([128, E], f32, tag="rL")
            nc.vector.tensor_copy(L[:], ps[:])
            Ssig = work.tile([128, E], f32, tag="rS")
            nc.scalar.activation(Ssig[:], ps[:], Act.Sigmoid)
            Sb = work.tile([128, E], f32, tag="rSb")
            nc.vector.tensor_tensor(Sb[:], Ssig[:], ebias_b[:], op=Alu.add)

            # group score = top-2 sum per group = max over pair sums
            Sv = Sb[:].rearrange("p (g i) -> p g i", i=GS)
            gs = work.tile([128, NG], f32, tag="rGS")
            tmp = work.tile([128, NG], f32, tag="rtmp")
            nc.vector.tensor_tensor(gs[:], Sv[:, :, 0], Sv[:, :, 1], op=Alu.add)
            for (a, b) in [(0, 2), (0, 3), (1, 2), (1, 3), (2, 3)]:
                nc.vector.tensor_tensor(
                    tmp[:], Sv[:, :, a], Sv[:, :, b], op=Alu.add)
                nc.vector.tensor_tensor(gs[:], gs[:], tmp[:], op=Alu.max)

            m8g = work.tile([128, 8], f32, tag="rm8g")
            nc.vector.max(m8g[:], gs[:])
            gmask = work.tile([128, NG], f32, tag="rgm")
            nc.vector.tensor_scalar(
                gmask[:], gs[:], m8g[:, TKG - 1:TKG], None, op0=Alu.is_ge)

            emask = work.tile([128, E], f32, tag="rem")
            emv = emask[:].rearrange("p (g i) -> p g i", i=GS)
            for r in range(GS):
                nc.vector.tensor_copy(emv[:, :, r], gmask[:])

            # top-4 experts among unmasked, compared on exact logits
            emask8 = work.tile([128, E], dt.uint8, tag="rem8")
            nc.vector.tensor_copy(emask8[:], emask[:])
            ml = work.tile([128, E], f32, tag="rml")
            nc.vector.tensor_copy(ml[:], negbuf[:])
            nc.vector.copy_predicated(ml[:], emask8[:], L[:])
            m8e = work.tile([128, 8], f32, tag="rm8e")
            nc.vector.max(m8e[:], ml[:])
            sel = work.tile([128, E], f32, tag="rsel")
            nc.vector.tensor_scalar(
                sel[:], ml[:], m8e[:, K - 1:K], None, op0=Alu.is_ge)

            wm = work.tile([128, E], f32, tag="rwm")
            nc.vector.tensor_tensor(wm[:], Ssig[:], sel[:], op=Alu.mult)
            den = work.tile([128, 1], f32, tag="rden")
            nc.vector.tensor_reduce(
                den[:], wm[:], axis=mybir.AxisListType.X, op=Alu.add)
            nc.vector.tensor_scalar(den[:], den[:], 1e-20, None, op0=Alu.add)
            winv = work.tile([128, 1], f32, tag="rwinv")
            nc.vector.reciprocal(winv[:], den[:])

            sw = work.tile([128, 2 * E], f32, tag="rsw")
            nc.vector.tensor_copy(sw[:, 0:E], sel[:])
            nc.vector.tensor_scalar(
                sw[:, E:2 * E], wm[:], winv[:, 0:1], SCALE,
                op0=Alu.mult, op1=Alu.mult)
            nc.sync.dma_start(selw_own[tt * 128:(tt + 1) * 128, :], sw[:])

        # wdt copy of xT for the shared-expert GEMMs
        xTw = xtp.tile([128, DC, TL], wdt)
        for dc in range(DC):
            nc.vector.tensor_copy(xTw[:, dc, :], xT[:, dc, :])

        # ---------------- P3: AllGather routing ----------------
        nc.gpsimd.collective_compute(
            "AllGather", Alu.bypass, replica_groups=groups,
            ins=[selw_own[:]], outs=[selw_all[:]])

        # ---------------- P8a: shared expert gate/up (independent) --------
        HsT = persist.tile([128, MC, TL], wdt)
        for mc in range(MC):
            sg_t = wstream.tile([128, DC, 128], wdt, tag="wst")
            off = OFF_SHGU + mc * 2 * SG_COLS
            nc.sync.dma_start(
                sg_t[:],
                wcat[:, off:off + SG_COLS].rearrange(
                    "p (c i) -> p c i", i=128))
            su_t = wstream.tile([128, DC, 128], wdt, tag="wst2")
            nc.sync.dma_start(
                su_t[:],
                wcat[:, off + SG_COLS:off + 2 * SG_COLS].rearrange(
                    "p (c i) -> p c i", i=128))
            pg = psum_g.tile([128, TL], f32, tag="pg")
            pu = psum_u.tile([128, TL], f32, tag="pu")
            for dc in range(DC):
                nc.tensor.matmul(
                    pg[:], sg_t[:, dc, :], xTw[:, dc, :],
                    start=(dc == 0), stop=(dc == DC - 1))
            for dc in range(DC):
                nc.tensor.matmul(
                    pu[:], su_t[:, dc, :], xTw[:, dc, :],
                    start=(dc == 0), stop=(dc == DC - 1))
            sig = work.tile([128, TL], f32, tag="ssig")
            nc.scalar.activation(sig[:], pg[:], Act.Sigmoid)
            sil = work.tile([128, TL], wdt, tag="ssil")
            nc.vector.tensor_tensor(sil[:], sig[:], pg[:], op=Alu.mult)
            nc.vector.tensor_tensor(HsT[:, mc, :], sil[:], pu[:], op=Alu.mult)

        xtp_cm.__exit__(None, None, None)

        # ---------------- P4: routing tables (chunked over tokens) --------
        sgin_t = persist.tile([16, EL, T // 16], f32)
        sgin_w = persist.tile([16, EL, T // 16], f32)
        carry = persist.tile([EL, 1], f32)
        nc.gpsimd.memset(carry[:], 0.0)

        route_cm = tc.tile_pool(name="route", bufs=1)
        route = route_cm.__enter__()
        for q in range(CH):
            selwT_c = route.tile([64, CT // 128, 128], f32, tag="selwT")
            for j in range(CT // 128):
                tt = q * (CT // 128) + j
                swt = work.tile([128, 2 * E], f32, tag="swt")
                nc.sync.dma_start(
                    swt[:], selw_all[tt * 128:(tt + 1) * 128, :])
                pt = psum_t.tile([64, 128], f32, tag="pt")
                nc.tensor.transpose(pt[:], swt[:], ident[:])
                nc.vector.tensor_copy(selwT_c[:, j, :], pt[:])

            SW_sel = route.tile([EL, CT], f32, tag="SWsel")
            SW_w = route.tile([EL, CT], f32, tag="SWw")
            for h in range(CT // 512):
                pswl = psum_g.tile([36, 512], f32, tag="pg")
                nc.tensor.matmul(
                    pswl[:], sloc_sb[:], selwT_c[:, 4 * h:4 * (h + 1), :],
                    start=True, stop=True)
                nc.vector.tensor_copy(
                    SW_sel[:, h * 512:(h + 1) * 512], pswl[0:EL, :])
                nc.vector.tensor_copy(
                    SW_w[:, h * 512:(h + 1) * 512], pswl[32:36, :])

            rank_c = route.tile([EL, CT], f32, tag="rankc")
            nc.vector.tensor_tensor_scan(
                rank_c[:], SW_sel[:], SW_sel[:], carry[:, 0:1],
                op0=Alu.add, op1=Alu.bypass)
            nc.vector.tensor_copy(carry[:], rank_c[:, CT - 1:CT])

            fsel_c = route.tile([EL, CT], f32, tag="fselc")
            nc.vector.tensor_scalar(
                fsel_c[:], rank_c[:], float(C), None, op0=Alu.is_le)
            nc.vector.tensor_tensor(
                fsel_c[:], fsel_c[:], SW_sel[:], op=Alu.mult)

            iota_i = route.tile([EL, CT], dt.int32, tag="iotai")
            nc.gpsimd.iota(iota_i[:], pattern=[[1, CT]], base=1 + q * CT,
                           channel_multiplier=0)
            iota_f = route.tile([EL, CT], f32, tag="iotaf")
            nc.vector.tensor_copy(iota_f[:], iota_i[:])

            At_c = route.tile([EL, CT], f32, tag="Atc")
            nc.vector.tensor_tensor(At_c[:], fsel_c[:], iota_f[:], op=Alu.mult)
            nc.vector.tensor_scalar(At_c[:], At_c[:], 1.0, None,
                                    op0=Alu.subtract)

            fsel8 = route.tile([EL, CT], dt.uint8, tag="fsel8")
            nc.vector.tensor_copy(fsel8[:], fsel_c[:])
            Aw_c = route.tile([EL, CT], f32, tag="Awc")
            nc.gpsimd.memset(Aw_c[:], -1.0)
            nc.vector.copy_predicated(Aw_c[:], fsel8[:], SW_w[:])

            nc.sync.dma_start(at_dram[:, q * CT:(q + 1) * CT], At_c[:])
            nc.sync.dma_start(aw_dram[:, q * CT:(q + 1) * CT], Aw_c[:])

        for e in range(EL):
            nc.sync.dma_start(
                sgin_t[:, e, :],
                at_dram[e].rearrange("(c b) -> b c", b=16))
            nc.sync.dma_start(
                sgin_w[:, e, :],
                aw_dram[e].rearrange("(c b) -> b c", b=16))

        # per-expert compaction -> slot lists + weights
        idx16s = persist.tile([16, EL * 16], dt.int16)   # 16-row wrapped
        idx16 = persist.tile([128, EL * 16], dt.int16)   # replicated to 128
        w_col = persist.tile([128, 2 * EL], f32)

        sgtoks, sgws = [], []
        for e in range(EL):
            sgtok = work.tile([16, 16], f32, tag=f"sgtok{e}")
            nft = work.tile([1, 1], dt.uint32, tag=f"nft{e}")
            nc.gpsimd.sparse_gather(sgtok[:], sgin_t[:, e, :], num_found=nft[:])
            sgw = work.tile([16, 16], f32, tag=f"sgw{e}")
            nfw = work.tile([1, 1], dt.uint32, tag=f"nfw{e}")
            nc.gpsimd.sparse_gather(sgw[:], sgin_w[:, e, :], num_found=nfw[:])
            nf_f = work.tile([1, 1], f32, tag=f"nf_f{e}")
            nc.vector.tensor_copy(nf_f[:], nft[:])
            nc.sync.dma_start(nf_dram[0:1, e:e + 1], nf_f[:])
            sgtoks.append(sgtok)
            sgws.append(sgw)

        for e in range(EL):
            sgtok, sgw = sgtoks[e], sgws[e]
            nf16 = work.tile([16, 1], f32, tag=f"nf16{e}")
            nc.sync.dma_start(
                nf16[:], nf_dram[0:1, e:e + 1].broadcast_to([16, 1]))
            vm = work.tile([16, 16], f32, tag=f"vm{e}")
            nc.vector.tensor_scalar(
                vm[:], iota16[:], nf16[:, 0:1], None, op0=Alu.is_lt)

            tokm = work.tile([16, 16], f32, tag=f"tokm{e}")
            nc.vector.tensor_tensor(tokm[:], sgtok[:], vm[:], op=Alu.mult)
            nc.vector.tensor_copy(idx16s[:, 16 * e:16 * (e + 1)], tokm[:])

            wsl = work.tile([16, 16], f32, tag=f"wsl{e}")
            nc.vector.tensor_tensor(wsl[:], sgw[:], vm[:], op=Alu.mult)
            ptw = psum_t.tile([16, 16], f32, tag="pt")
            nc.tensor.transpose(ptw[:], wsl[:], ident[:16, :16])
            wt16 = work.tile([16, 16], f32, tag=f"wt16{e}")
            nc.vector.tensor_copy(wt16[:], ptw[:])
            nc.sync.dma_start(w_col[:, 2 * e:2 * e + 1], wt16[0:8, :])
            nc.sync.dma_start(w_col[:, 2 * e + 1:2 * e + 2], wt16[8:16, :])

        # replicate the 16-row wrapped index block to all 128 partitions
        nc.sync.dma_start(idx_dram[:], idx16s[:])
        for r in range(8):
            nc.sync.dma_start(idx16[16 * r:16 * (r + 1), :], idx_dram[:])

        route_cm.__exit__(None, None, None)

        # ---------------- P5..P7: dispatch + routed expert GEMMs ----------
        with tc.tile_pool(name="dpXP", bufs=2) as dpXP, \
                tc.tile_pool(name="dpXPT", bufs=1) as dpXPT, \
                tc.tile_pool(name="dpHT", bufs=1) as dpHT, \
                tc.tile_pool(name="dpWD", bufs=1) as dpWD:
            for e in range(EL):
                eoff = OFF_EXP + e * EXP_STRIDE
                XP = dpXP.tile([128, CP // 128, D], wdt, tag="XP")
                nc.gpsimd.dma_gather(
                    XP[:], x_full[:], idx16[:, 16 * e:16 * (e + 1)],
                    CP, CP, D, queue_num=0)
                XPT = dpXPT.tile([128, DC, CPV], wdt, tag="XPT")
                for col in range(CP // 128):
                    cw = min(128, CPV - col * 128)
                    for dc in range(DC):
                        ptx = psum_t.tile([128, 128], wdt, tag="pt")
                        nc.tensor.transpose(
                            ptx[:], XP[:, col, dc * 128:(dc + 1) * 128],
                            ident_w[:])
                        nc.vector.tensor_copy(
                            XPT[:, dc, col * 128:col * 128 + cw],
                            ptx[:, 0:cw])

                HT = dpHT.tile([128, IC, CPV], wdt, tag="HT")
                for ic in range(IC):
                    goff = eoff + ic * 2 * WG_COLS
                    wg_t = wstream.tile([128, DC, 128], wdt, tag="wst")
                    nc.sync.dma_start(
                        wg_t[:],
                        wcat[:, goff:goff + WG_COLS].rearrange(
                            "p (c i) -> p c i", i=128))
                    wu_t = wstream.tile([128, DC, 128], wdt, tag="wst2")
                    nc.sync.dma_start(
                        wu_t[:],
                        wcat[:, goff + WG_COLS:goff + 2 * WG_COLS].rearrange(
                            "p (c i) -> p c i", i=128))
                    pg = psum_g.tile([128, CPV], f32, tag="pg")
                    pu = psum_u.tile([128, CPV], f32, tag="pu")
                    for dc in range(DC):
                        nc.tensor.matmul(
                            pg[:], wg_t[:, dc, :], XPT[:, dc, :],
                            start=(dc == 0), stop=(dc == DC - 1))
                    for dc in range(DC):
                        nc.tensor.matmul(
                            pu[:], wu_t[:, dc, :], XPT[:, dc, :],
                            start=(dc == 0), stop=(dc == DC - 1))
                    sig = work.tile([128, CPV], f32, tag="esig")
                    nc.scalar.activation(sig[:], pg[:], Act.Sigmoid)
                    sil = work.tile([128, CPV], wdt, tag="esil")
                    nc.vector.tensor_tensor(sil[:], sig[:], pg[:], op=Alu.mult)
                    nc.vector.tensor_tensor(
                        HT[:, ic, :], sil[:], pu[:], op=Alu.mult)

                for dc4 in range(D // 512):
                    doff = eoff + IC * 2 * WG_COLS + dc4 * WD_COLS
                    wd_t = dpWD.tile([128, IC, 512], wdt, tag="wst3")
                    nc.sync.dma_start(
                        wd_t[:],
                        wcat[:, doff:doff + WD_COLS].rearrange(
                            "p (c d) -> p c d", d=512))
                    for sb in range(2):
                        sw = min(128, CPV - sb * 128)
                        py = psum_y.tile([128, 512], f32, tag="py")
                        for ic in range(IC):
                            nc.tensor.matmul(
                                py[0:sw, :],
                                HT[:, ic, sb * 128:sb * 128 + sw],
                                wd_t[:, ic, :],
                                start=(ic == 0), stop=(ic == IC - 1))
                        yw = work.tile([128, 512], pdt, tag="yw")
                        if sw < 128:
                            nc.gpsimd.memset(yw[sw:128, :], 0.0)
                        nc.vector.tensor_scalar(
                            yw[0:sw, :], py[0:sw, :],
                            w_col[0:sw, 2 * e + sb:2 * e + sb + 1], None,
                            op0=Alu.mult)
                        nidx = sw // 16
                        nc.gpsimd.dma_scatter_add(
                            partials[dc4][:],
                            yw[:].rearrange("p (a f) -> p a f", a=1),
                            idx16[:, 16 * e + 8 * sb:
                                  16 * e + 8 * sb + nidx],
                            sw, sw, 512, elem_step=512, queue_num=0)
                    # interleaved chunked ReduceScatter: partial column j
                    # is complete once the last expert has scattered it
                    if e == EL - 1:
                        nc.gpsimd.collective_compute(
                            "ReduceScatter", Alu.add, replica_groups=groups,
                            ins=[partials[dc4][:]], outs=[rs_outs[dc4][:]])

        # ---------------- P8b: shared expert down proj (during RS) --------
        # computed fully into SBUF so no DMA in this phase waits on the
        # ReduceScatter; only the final combine below depends on rs_out.
        shr_sb = persist.tile([128, D // 512, TT, 512], f32)
        with tc.tile_pool(name="sdpool", bufs=2) as sdpool:
            for dc4 in range(D // 512):
                off = OFF_SHD + dc4 * SD_COLS
                sd_t = sdpool.tile([128, MC, 512], wdt, tag="wsd")
                nc.sync.dma_start(
                    sd_t[:],
                    wcat[:, off:off + SD_COLS].rearrange(
                        "p (c d) -> p c d", d=512))
                for tb in range(TT):
                    po = psum_y.tile([128, 512], f32, tag="py")
                    for mc in range(MC):
                        nc.tensor.matmul(
                            po[:], HsT[:, mc, tb * 128:(tb + 1) * 128],
                            sd_t[:, mc, :],
                            start=(mc == 0), stop=(mc == MC - 1))
                    nc.vector.tensor_copy(shr_sb[:, dc4, tb, :], po[:])

        # ---------------- P9b: final combine (waits on RS) ----------------
        for dc4 in range(D // 512):
            for tb in range(TT):
                rst = work.tile([128, 512], pdt, tag="rst")
                nc.sync.dma_start(
                    rst[:], rs_outs[dc4][tb * 128:(tb + 1) * 128, :])
                fin = work.tile([128, 512], f32, tag="fin")
                nc.vector.tensor_tensor(
                    fin[:], shr_sb[:, dc4, tb, :], rst[:], op=Alu.add)
                nc.sync.dma_start(
                    out[tb * 128:(tb + 1) * 128,
                        dc4 * 512:(dc4 + 1) * 512], fin[:])

    nc.finalize()
    return nc


_NC_CACHE = {}


def get_nc(gemm_mode=None):
    gemm_mode = gemm_mode or GEMM_MODE
    if gemm_mode not in _NC_CACHE:
        _NC_CACHE[gemm_mode] = _build(gemm_mode)
    return _NC_CACHE[gemm_mode]


def make_in_maps(inputs, gemm_mode=None):
    """Shard + pre-lay-out full inputs into the 8 per-core input maps."""
    import ml_dtypes

    gemm_mode = gemm_mode or GEMM_MODE
    wnp = ml_dtypes.bfloat16 if gemm_mode == "bf16" else np.float32

    x = np.asarray(inputs["hidden_states"], np.float32).reshape(T, D)
    router_w = np.asarray(inputs["router_w"], np.float32)
    e_bias = np.asarray(inputs["e_bias"], np.float32).reshape(E)
    W_gate = np.asarray(inputs["W_gate"], np.float32)
    W_up = np.asarray(inputs["W_up"], np.float32)
    W_down = np.asarray(inputs["W_down"], np.float32)
    shared_gate = np.asarray(inputs["shared_gate"], np.float32)
    shared_up = np.asarray(inputs["shared_up"], np.float32)
    shared_down = np.asarray(inputs["shared_down"], np.float32)

    x_w = np.ascontiguousarray(x).astype(wnp)

    # shared blob pieces (identical on every core)
    shgu = np.empty((128, MC, 2, SG_COLS), np.float32)
    # sg_t[p, dc, j] = shared_gate[mc*128+j, dc*128+p]
    sg4 = shared_gate.reshape(MC, 128, DC, 128)      # [mc, j, dc, p]
    su4 = shared_up.reshape(MC, 128, DC, 128)
    shgu[:, :, 0, :] = np.transpose(sg4, (3, 0, 2, 1)).reshape(128, MC, -1)
    shgu[:, :, 1, :] = np.transpose(su4, (3, 0, 2, 1)).reshape(128, MC, -1)
    shgu = shgu.reshape(128, -1)
    # sd_t[p, mc, d] = shared_down[dc4*512+d, mc*128+p]
    sd4 = shared_down.reshape(4, 512, MC, 128)       # [dc4, d, mc, p]
    shd = np.transpose(sd4, (3, 0, 2, 1)).reshape(128, -1)

    # router small blob (identical on every core except sloc)
    # rwT_sb[p, dc, e] = router_w[e, dc*128+p]
    rw3 = router_w.reshape(E, DC, 128)               # [e, dc, p]
    rwL = np.transpose(rw3, (2, 1, 0)).reshape(128, -1)
    eb = np.broadcast_to(e_bias[None, :], (128, E))

    in_maps = []
    for c in range(NCORES):
        sl = np.zeros((128, 36), np.float32)
        for j in range(EL):
            sl[EL * c + j, j] = 1.0
            sl[E + EL * c + j, 32 + j] = 1.0
        scat = np.concatenate([rwL, eb, sl], axis=1).astype(np.float32)

        # xT_in[p, dc*TL + t] = x[c*TL + t, dc*128 + p]
        xo = x[TL * c:TL * (c + 1)]                  # [TL, D]
        xo3 = xo.reshape(TL, DC, 128)                # [t, dc, p]
        xT_in = np.ascontiguousarray(
            np.transpose(xo3, (2, 1, 0)).reshape(128, -1))

        # expert blob
        eparts = []
        for e in range(EL):
            ge = EL * c + e
            # wg_t[p, dc, i2] for ic: = W_gate[ge][dc*128+p, ic*128+i2]
            wg4 = W_gate[ge].reshape(DC, 128, IC, 128)   # [dc, p, ic, i2]
            wu4 = W_up[ge].reshape(DC, 128, IC, 128)
            gu = np.empty((128, IC, 2, WG_COLS), np.float32)
            gu[:, :, 0, :] = np.transpose(
                wg4, (1, 2, 0, 3)).reshape(128, IC, -1)
            gu[:, :, 1, :] = np.transpose(
                wu4, (1, 2, 0, 3)).reshape(128, IC, -1)
            eparts.append(gu.reshape(128, -1))
            # wd_t[p, ic, d] for dc4: = W_down[ge][ic*128+p, dc4*512+d]
            wd4 = W_down[ge].reshape(IC, 128, 4, 512)    # [ic, p, dc4, d]
            eparts.append(np.transpose(
                wd4, (1, 2, 0, 3)).reshape(128, -1))
        wcat = np.concatenate([shgu, shd] + eparts, axis=1).astype(wnp)
        assert wcat.shape[1] == WCOLS, wcat.shape

        in_maps.append({
            "x_full": x_w,
            "xT_in": xT_in,
            "wcat": np.ascontiguousarray(wcat),
            "scat": np.ascontiguousarray(scat),
        })
    return in_maps


def kernel(**inputs):
    from concourse.bass_utils import run_bass_kernel_spmd

    nc = get_nc()
    in_maps = make_in_maps(inputs)
    res = run_bass_kernel_spmd(
        nc, in_maps, core_ids=list(range(NCORES)), trace=False)
    out = np.concatenate([res.results[c]["out"] for c in range(NCORES)], axis=0)
    return out.reshape(B, S, D)


kernel.last_exec_time_ns = None


# revision 12
# speedup vs baseline: 11.0099x; 2.5257x over previous
"""DeepseekV3 MoE layer on 8 Trainium2 NeuronCores (Bass/Tile), v2.

Sharding (same topology as v1):
  - Router: data-parallel (each core routes its own T/8=512 tokens, fp32,
    selection done on exact logits), then AllGather of per-token
    (sel-mask, weight) -> every core knows the full routing.
  - Routed experts: expert-parallel, 4 experts/core; dma_gather rows,
    GEMM, dma_scatter_add into a [T, D] partial.
  - Combine: ReduceScatter(add) + locally computed shared-expert MLP.

v2 changes vs v1:
  - Heavy GEMMs in bf16 (or f32r): 4x PE throughput; router stays fp32.
  - All weights pre-laid out on host into ONE contiguous [128, COLS]
    blob in exact kernel read order -> every weight DMA is a wide
    contiguous column slice (>=4KB per partition line).
  - x transposed on host -> P1 on-device transpose removed.
  - Inputs packed: 4 tensors/core instead of 11 (less per-exec staging).
  - ReduceScatter issued before the shared-expert down-projection so the
    collective overlaps PE work.

kernel(**inputs) takes the full unsharded inputs and returns the full
[B, S, D] output.  Self-contained: hardcodes all shapes.
"""

import os
import sys

for _p in ("/opt/trn_rl_repo", "/opt/pypackages"):
    if _p not in sys.path:
        sys.path.insert(0, _p)

import numpy as np

# ---------------------------------------------------------------- constants
B, S, D = 2, 2048, 2048
T = B * S                  # 4096 tokens
I = 1024                   # routed expert intermediate
E = 32                     # routed experts
K = 4                      # experts per token
NG = 8                     # groups
GS = E // NG               # experts per group = 4
TKG = 3                    # top-k groups
ISH = 2048                 # shared expert intermediate (I * n_shared)
SCALE = 2.5
C = 160                    # capacity = ceil(1.25 * T / E)
CP = 256                   # per-expert slot padding (128-aligned)
CPV = 192                  # processed slots per expert (>= C, 64-multiple)
NCORES = 8
EL = E // NCORES           # local experts per core = 4
TL = T // NCORES           # local tokens per core = 512
NSLOT = EL * CP            # padded slots per core = 1024

DC = D // 128              # 16 d-chunks
IC = I // 128              # 8  i-chunks
MC = ISH // 128            # 16 shared-intermediate chunks
TT = TL // 128             # 4 own-token tiles
NT = T // 128              # 32 all-token tiles
CH = 4                     # routing-table token chunks
CT = T // CH               # 1024 tokens per chunk

# weight blob column offsets (in elements of wdt), [128, WCOLS]
# order: shared gate/up (per mc), shared down (per dc4),
#        per expert: gate/up (per ic), down (per dc4)
SG_COLS = DC * 128         # 2048 per mc (gate), same for up
SD_COLS = MC * 512         # 8192 per dc4
WG_COLS = DC * 128         # 2048 per (e, ic)
WD_COLS = IC * 512         # 4096 per (e, dc4)
OFF_SHGU = 0                                   # mc-major: [gate, up] pairs
OFF_SHD = OFF_SHGU + MC * 2 * SG_COLS          # 65536
OFF_EXP = OFF_SHD + 4 * SD_COLS                # 98304
EXP_STRIDE = IC * 2 * WG_COLS + 4 * WD_COLS    # 49152 per expert
WCOLS = OFF_EXP + EL * EXP_STRIDE              # 294912

# f32 small blob: [128, SCOLS]: rwT (DC*E) | ebias (E) | sloc (36)
RW_COLS = DC * E           # 512
OFF_EB = RW_COLS
OFF_SL = RW_COLS + E
SCOLS = RW_COLS + E + 36   # 580

# "bf16" | "f32r" : dtype of the heavy GEMMs (router stays f32)
GEMM_MODE = os.environ.get("BASS_MOE_GEMM_MODE", "bf16")
# dtype of the partial-sum buffer + ReduceScatter ("bf16" | "f32")
PART_DT = os.environ.get("BASS_MOE_PART_DT", "bf16")


# ---------------------------------------------------------------- builder
def _build(gemm_mode: str):
    import concourse.bass as bass
    import concourse.bacc as bacc
    import concourse.mybir as mybir
    import concourse.tile as tile
    from concourse import masks
    from contextlib import ExitStack

    dt = mybir.dt
    Alu = mybir.AluOpType
    Act = mybir.ActivationFunctionType

    f32 = dt.float32
    wdt = dt.bfloat16 if gemm_mode == "bf16" else dt.float32r
    pdt = dt.bfloat16 if PART_DT == "bf16" else f32

    nc = bacc.Bacc(None, num_devices=NCORES, num_swdge_queues=1)
    groups = [list(range(NCORES))]

    # ---------------- I/O ----------------
    x_full = nc.dram_tensor("x_full", [T, D], wdt, kind="ExternalInput")
    xT_in = nc.dram_tensor("xT_in", [128, DC * TL], f32, kind="ExternalInput")
    wcat = nc.dram_tensor("wcat", [128, WCOLS], wdt, kind="ExternalInput")
    scat = nc.dram_tensor("scat", [128, SCOLS], f32, kind="ExternalInput")
    out = nc.dram_tensor("out", [TL, D], f32, kind="ExternalOutput")

    # ---------------- internal DRAM ----------------
    selw_own = nc.dram_tensor("selw_own", [TL, 2 * E], f32)
    selw_all = nc.dram_tensor("selw_all", [T, 2 * E], f32, addr_space="Shared")
    partials = [nc.dram_tensor(f"partial{j}", [T, 512], pdt)
                for j in range(D // 512)]
    rs_outs = [nc.dram_tensor(f"rs_out{j}", [TL, 512], pdt)
               for j in range(D // 512)]
    at_dram = nc.dram_tensor("at_dram", [EL, T], f32)
    nf_dram = nc.dram_tensor("nf_dram", [1, EL], f32)
    aw_dram = nc.dram_tensor("aw_dram", [EL, T], f32)

    with tile.TileContext(nc) as tc, ExitStack() as ctx:
        consts = ctx.enter_context(tc.tile_pool(name="consts", bufs=1))
        work = ctx.enter_context(tc.tile_pool(name="work", bufs=2))
        psum_t = ctx.enter_context(
            tc.tile_pool(name="psum_t", bufs=2, space="PSUM"))
        psum_g = ctx.enter_context(
            tc.tile_pool(name="psum_g", bufs=2, space="PSUM"))
        psum_u = ctx.enter_context(
            tc.tile_pool(name="psum_u", bufs=2, space="PSUM"))
        psum_y = ctx.enter_context(
            tc.tile_pool(name="psum_y", bufs=2, space="PSUM"))
        persist = ctx.enter_context(tc.tile_pool(name="persist", bufs=1))
        wstream = ctx.enter_context(tc.tile_pool(name="wstream", bufs=2))

        # ---------------- inputs -> SBUF ----------------
        scat_sb = consts.tile([128, SCOLS], f32)
        nc.sync.dma_start(scat_sb[:], scat[:])
        rwT_sb = scat_sb[:, 0:RW_COLS].rearrange("p (c e) -> p c e", e=E)
        ebias_b = scat_sb[:, OFF_EB:OFF_EB + E]
        sloc_sb = scat_sb[0:64, OFF_SL:OFF_SL + 36]

        xtp_cm = tc.tile_pool(name="xtp", bufs=1)
        xtp = xtp_cm.__enter__()
        xT = xtp.tile([128, DC, TL], f32)
        nc.sync.dma_start(
            xT[:], xT_in[:].rearrange("p (c t) -> p c t", t=TL))

        # ---------------- constants ----------------
        ident = consts.tile([128, 128], f32)
        masks.make_identity(nc, ident[:])
        ident_w = consts.tile([128, 128], wdt)
        nc.vector.tensor_copy(ident_w[:], ident[:])

        negbuf = consts.tile([128, E], f32)
        nc.gpsimd.memset(negbuf[:], -1e30)

        iota16_i = consts.tile([16, 16], dt.int32)
        nc.gpsimd.iota(iota16_i[:], pattern=[[16, 16]], base=0,
                       channel_multiplier=1)
        iota16 = consts.tile([16, 16], f32)
        nc.vector.tensor_copy(iota16[:], iota16_i[:])

        # zero-fill the [T, D] partial (overlaps with router/shared gemm)
        zt = consts.tile([128, 512], pdt)
        nc.gpsimd.memset(zt[:], 0.0)
        for r in range(NT):
            for zc in range(D // 512):
                nc.scalar.dma_start(
                    partials[zc][r * 128:(r + 1) * 128, :], zt[:])

        # ---------------- P2: router on own tokens (fp32/exact) -----------
        for tt in range(TT):
            ps = psum_t.tile([128, E], f32, tag="pt")
            for dc in range(DC):
                nc.tensor.matmul(
                    ps[:], xT[:, dc, tt * 128:(tt + 1) * 128], rwT_sb[:, dc, :],
                    start=(dc == 0), stop=(dc == DC - 1))
            L = work.tile([128, E], f32, tag="rL")
            nc.vector.tensor_copy(L[:], ps[:])
            Ssig = work.tile([128, E], f32, tag="rS")
            nc.scalar.activation(Ssig[:], ps[:], Act.Sigmoid)
            Sb = work.tile([128, E], f32, tag="rSb")
            nc.vector.tensor_tensor(Sb[:], Ssig[:], ebias_b[:], op=Alu.add)

            # group score = top-2 sum per group = max over pair sums
            Sv = Sb[:].rearrange("p (g i) -> p g i", i=GS)
            gs = work.tile([128, NG], f32, tag="rGS")
            tmp = work.tile([128, NG], f32, tag="rtmp")
            nc.vector.tensor_tensor(gs[:], Sv[:, :, 0], Sv[:, :, 1], op=Alu.add)
            for (a, b) in [(0, 2), (0, 3), (1, 2), (1, 3), (2, 3)]:
                nc.vector.tensor_tensor(
                    tmp[:], Sv[:, :, a], Sv[:, :, b], op=Alu.add)
                nc.vector.tensor_tensor(gs[:], gs[:], tmp[:], op=Alu.max)

            m8g = work.tile([128, 8], f32, tag="rm8g")
            nc.vector.max(m8g[:], gs[:])
            gmask = work.tile([128, NG], f32, tag="rgm")
            nc.vector.tensor_scalar(
                gmask[:], gs[:], m8g[:, TKG - 1:TKG], None, op0=Alu.is_ge)

            emask = work.tile([128, E], f32, tag="rem")
            emv = emask[:].rearrange("p (g i) -> p g i", i=GS)
            for r in range(GS):
                nc.vector.tensor_copy(emv[:, :, r], gmask[:])

            # top-4 experts among unmasked, compared on exact logits
            emask8 = work.tile([128, E], dt.uint8, tag="rem8")
            nc.vector.tensor_copy(emask8[:], emask[:])
            ml = work.tile([128, E], f32, tag="rml")
            nc.vector.tensor_copy(ml[:], negbuf[:])
            nc.vector.copy_predicated(ml[:], emask8[:], L[:])
            m8e = work.tile([128, 8], f32, tag="rm8e")
            nc.vector.max(m8e[:], ml[:])
            sel = work.tile([128, E], f32, tag="rsel")
            nc.vector.tensor_scalar(
                sel[:], ml[:], m8e[:, K - 1:K], None, op0=Alu.is_ge)

            wm = work.tile([128, E], f32, tag="rwm")
            nc.vector.tensor_tensor(wm[:], Ssig[:], sel[:], op=Alu.mult)
            den = work.tile([128, 1], f32, tag="rden")
            nc.vector.tensor_reduce(
                den[:], wm[:], axis=mybir.AxisListType.X, op=Alu.add)
            nc.vector.tensor_scalar(den[:], den[:], 1e-20, None, op0=Alu.add)
            winv = work.tile([128, 1], f32, tag="rwinv")
            nc.vector.reciprocal(winv[:], den[:])

            sw = work.tile([128, 2 * E], f32, tag="rsw")
            nc.vector.tensor_copy(sw[:, 0:E], sel[:])
            nc.vector.tensor_scalar(
                sw[:, E:2 * E], wm[:], winv[:, 0:1], SCALE,
                op0=Alu.mult, op1=Alu.mult)
            nc.sync.dma_start(selw_own[tt * 128:(tt + 1) * 128, :], sw[:])

        # wdt copy of xT for the shared-expert GEMMs
        xTw = xtp.tile([128, DC, TL], wdt)
        for dc in range(DC):
            nc.vector.tensor_copy(xTw[:, dc, :], xT[:, dc, :])

        # ---------------- P3: AllGather routing ----------------
        nc.gpsimd.collective_compute(
            "AllGather", Alu.bypass, replica_groups=groups,
            ins=[selw_own[:]], outs=[selw_all[:]])

        # ---------------- P8a: shared expert gate/up (independent) --------
        HsT = persist.tile([128, MC, TL], wdt)
        for mc in range(MC):
            sg_t = wstream.tile([128, DC, 128], wdt, tag="wst")
            off = OFF_SHGU + mc * 2 * SG_COLS
            nc.sync.dma_start(
                sg_t[:],
                wcat[:, off:off + SG_COLS].rearrange(
                    "p (c i) -> p c i", i=128))
            su_t = wstream.tile([128, DC, 128], wdt, tag="wst2")
            nc.sync.dma_start(
                su_t[:],
                wcat[:, off + SG_COLS:off + 2 * SG_COLS].rearrange(
                    "p (c i) -> p c i", i=128))
            pg = psum_g.tile([128, TL], f32, tag="pg")
            pu = psum_u.tile([128, TL], f32, tag="pu")
            for dc in range(DC):
                nc.tensor.matmul(
                    pg[:], sg_t[:, dc, :], xTw[:, dc, :],
                    start=(dc == 0), stop=(dc == DC - 1))
            for dc in range(DC):
                nc.tensor.matmul(
                    pu[:], su_t[:, dc, :], xTw[:, dc, :],
                    start=(dc == 0), stop=(dc == DC - 1))
            sig = work.tile([128, TL], f32, tag="ssig")
            nc.scalar.activation(sig[:], pg[:], Act.Sigmoid)
            sil = work.tile([128, TL], wdt, tag="ssil")
            nc.vector.tensor_tensor(sil[:], sig[:], pg[:], op=Alu.mult)
            nc.vector.tensor_tensor(HsT[:, mc, :], sil[:], pu[:], op=Alu.mult)

        xtp_cm.__exit__(None, None, None)

        # ---------------- P4: routing tables (chunked over tokens) --------
        sgin_t = persist.tile([16, EL, T // 16], f32)
        sgin_w = persist.tile([16, EL, T // 16], f32)
        carry = persist.tile([EL, 1], f32)
        nc.gpsimd.memset(carry[:], 0.0)

        route_cm = tc.tile_pool(name="route", bufs=1)
        route = route_cm.__enter__()
        for q in range(CH):
            selwT_c = route.tile([64, CT // 128, 128], f32, tag="selwT")
            for j in range(CT // 128):
                tt = q * (CT // 128) + j
                swt = work.tile([128, 2 * E], f32, tag="swt")
                nc.sync.dma_start(
                    swt[:], selw_all[tt * 128:(tt + 1) * 128, :])
                pt = psum_t.tile([64, 128], f32, tag="pt")
                nc.tensor.transpose(pt[:], swt[:], ident[:])
                nc.vector.tensor_copy(selwT_c[:, j, :], pt[:])

            SW_sel = route.tile([EL, CT], f32, tag="SWsel")
            SW_w = route.tile([EL, CT], f32, tag="SWw")
            for h in range(CT // 512):
                pswl = psum_g.tile([36, 512], f32, tag="pg")
                nc.tensor.matmul(
                    pswl[:], sloc_sb[:], selwT_c[:, 4 * h:4 * (h + 1), :],
                    start=True, stop=True)
                nc.vector.tensor_copy(
                    SW_sel[:, h * 512:(h + 1) * 512], pswl[0:EL, :])
                nc.vector.tensor_copy(
                    SW_w[:, h * 512:(h + 1) * 512], pswl[32:36, :])

            rank_c = route.tile([EL, CT], f32, tag="rankc")
            nc.vector.tensor_tensor_scan(
                rank_c[:], SW_sel[:], SW_sel[:], carry[:, 0:1],
                op0=Alu.add, op1=Alu.bypass)
            nc.vector.tensor_copy(carry[:], rank_c[:, CT - 1:CT])

            fsel_c = route.tile([EL, CT], f32, tag="fselc")
            nc.vector.tensor_scalar(
                fsel_c[:], rank_c[:], float(C), None, op0=Alu.is_le)
            nc.vector.tensor_tensor(
                fsel_c[:], fsel_c[:], SW_sel[:], op=Alu.mult)

            iota_i = route.tile([EL, CT], dt.int32, tag="iotai")
            nc.gpsimd.iota(iota_i[:], pattern=[[1, CT]], base=1 + q * CT,
                           channel_multiplier=0)
            iota_f = route.tile([EL, CT], f32, tag="iotaf")
            nc.vector.tensor_copy(iota_f[:], iota_i[:])

            At_c = route.tile([EL, CT], f32, tag="Atc")
            nc.vector.tensor_tensor(At_c[:], fsel_c[:], iota_f[:], op=Alu.mult)
            nc.vector.tensor_scalar(At_c[:], At_c[:], 1.0, None,
                                    op0=Alu.subtract)

            fsel8 = route.tile([EL, CT], dt.uint8, tag="fsel8")
            nc.vector.tensor_copy(fsel8[:], fsel_c[:])
            Aw_c = route.tile([EL, CT], f32, tag="Awc")
            nc.gpsimd.memset(Aw_c[:], -1.0)
            nc.vector.copy_predicated(Aw_c[:], fsel8[:], SW_w[:])

            nc.sync.dma_start(at_dram[:, q * CT:(q + 1) * CT], At_c[:])
            nc.sync.dma_start(aw_dram[:, q * CT:(q + 1) * CT], Aw_c[:])

        for e in range(EL):
            nc.sync.dma_start(
                sgin_t[:, e, :],
                at_dram[e].rearrange("(c b) -> b c", b=16))
            nc.sync.dma_start(
                sgin_w[:, e, :],
                aw_dram[e].rearrange("(c b) -> b c", b=16))

        # per-expert compaction -> slot lists + weights
        idx16s = persist.tile([16, EL * 16], dt.int16)   # 16-row wrapped
        idx16 = persist.tile([128, EL * 16], dt.int16)   # replicated to 128
        w_col = persist.tile([128, 2 * EL], f32)

        sgtoks, sgws = [], []
        for e in range(EL):
            sgtok = work.tile([16, 16], f32, tag=f"sgtok{e}")
            nft = work.tile([1, 1], dt.uint32, tag=f"nft{e}")
            nc.gpsimd.sparse_gather(sgtok[:], sgin_t[:, e, :], num_found=nft[:])
            sgw = work.tile([16, 16], f32, tag=f"sgw{e}")
            nfw = work.tile([1, 1], dt.uint32, tag=f"nfw{e}")
            nc.gpsimd.sparse_gather(sgw[:], sgin_w[:, e, :], num_found=nfw[:])
            nf_f = work.tile([1, 1], f32, tag=f"nf_f{e}")
            nc.vector.tensor_copy(nf_f[:], nft[:])
            nc.sync.dma_start(nf_dram[0:1, e:e + 1], nf_f[:])
            sgtoks.append(sgtok)
            sgws.append(sgw)

        for e in range(EL):
            sgtok, sgw = sgtoks[e], sgws[e]
            nf16 = work.tile([16, 1], f32, tag=f"nf16{e}")
            nc.sync.dma_start(
                nf16[:], nf_dram[0:1, e:e + 1].broadcast_to([16, 1]))
            vm = work.tile([16, 16], f32, tag=f"vm{e}")
            nc.vector.tensor_scalar(
                vm[:], iota16[:], nf16[:, 0:1], None, op0=Alu.is_lt)

            tokm = work.tile([16, 16], f32, tag=f"tokm{e}")
            nc.vector.tensor_tensor(tokm[:], sgtok[:], vm[:], op=Alu.mult)
            nc.vector.tensor_copy(idx16s[:, 16 * e:16 * (e + 1)], tokm[:])

            wsl = work.tile([16, 16], f32, tag=f"wsl{e}")
            nc.vector.tensor_tensor(wsl[:], sgw[:], vm[:], op=Alu.mult)
            ptw = psum_t.tile([16, 16], f32, tag="pt")
            nc.tensor.transpose(ptw[:], wsl[:], ident[:16, :16])
            wt16 = work.tile([16, 16], f32, tag=f"wt16{e}")
            nc.vector.tensor_copy(wt16[:], ptw[:])
            nc.sync.dma_start(w_col[:, 2 * e:2 * e + 1], wt16[0:8, :])
            nc.sync.dma_start(w_col[:, 2 * e + 1:2 * e + 2], wt16[8:16, :])

        # replicate the 16-row wrapped index block to all 128 partitions
        # (direct SBUF->SBUF, no DRAM roundtrip)
        for r in range(8):
            nc.sync.dma_start(idx16[16 * r:16 * (r + 1), :], idx16s[:])

        route_cm.__exit__(None, None, None)

        # ---------------- P5..P7: dispatch + routed expert GEMMs ----------
        with tc.tile_pool(name="dpXPT", bufs=2) as dpXPT, \
                tc.tile_pool(name="dpHT", bufs=1) as dpHT, \
                tc.tile_pool(name="dpWD", bufs=2) as dpWD, \
                tc.tile_pool(name="ywpool", bufs=8) as ywpool:
            HTs = []
            for e in range(EL):
                eoff = OFF_EXP + e * EXP_STRIDE
                XPT = dpXPT.tile([128, DC, CP], wdt, tag="XPT")
                nc.gpsimd.dma_gather(
                    XPT[:], x_full[:], idx16[:, 16 * e:16 * (e + 1)],
                    CP, CP, D, transpose=True, queue_num=0)

                HT = dpHT.tile([128, IC, CPV], wdt, tag=f"HT{e}")
                HTs.append(HT)
                for ic in range(IC):
                    goff = eoff + ic * 2 * WG_COLS
                    wg_t = wstream.tile([128, DC, 128], wdt, tag="wst")
                    nc.sync.dma_start(
                        wg_t[:],
                        wcat[:, goff:goff + WG_COLS].rearrange(
                            "p (c i) -> p c i", i=128))
                    wu_t = wstream.tile([128, DC, 128], wdt, tag="wst2")
                    nc.sync.dma_start(
                        wu_t[:],
                        wcat[:, goff + WG_COLS:goff + 2 * WG_COLS].rearrange(
                            "p (c i) -> p c i", i=128))
                    pg = psum_g.tile([128, CPV], f32, tag="pg")
                    pu = psum_u.tile([128, CPV], f32, tag="pu")
                    for dc in range(DC):
                        nc.tensor.matmul(
                            pg[:], wg_t[:, dc, :], XPT[:, dc, 0:CPV],
                            start=(dc == 0), stop=(dc == DC - 1))
                    for dc in range(DC):
                        nc.tensor.matmul(
                            pu[:], wu_t[:, dc, :], XPT[:, dc, 0:CPV],
                            start=(dc == 0), stop=(dc == DC - 1))
                    sig = work.tile([128, CPV], f32, tag="esig")
                    nc.scalar.activation(sig[:], pg[:], Act.Sigmoid)
                    sil = work.tile([128, CPV], wdt, tag="esil")
                    nc.vector.tensor_tensor(sil[:], sig[:], pg[:], op=Alu.mult)
                    nc.vector.tensor_tensor(
                        HT[:, ic, :], sil[:], pu[:], op=Alu.mult)

            # down-projection swept dc4-outer so each partial column
            # completes early and its ReduceScatter chunk overlaps the rest
            for dc4 in range(D // 512):
                for e in range(EL):
                    eoff = OFF_EXP + e * EXP_STRIDE
                    doff = eoff + IC * 2 * WG_COLS + dc4 * WD_COLS
                    HT = HTs[e]
                    wd_t = dpWD.tile([128, IC, 512], wdt, tag="wst3")
                    nc.sync.dma_start(
                        wd_t[:],
                        wcat[:, doff:doff + WD_COLS].rearrange(
                            "p (c d) -> p c d", d=512))
                    for sb in range(2):
                        sw = min(128, CPV - sb * 128)
                        py = psum_y.tile([128, 512], f32, tag="py")
                        for ic in range(IC):
                            nc.tensor.matmul(
                                py[0:sw, :],
                                HT[:, ic, sb * 128:sb * 128 + sw],
                                wd_t[:, ic, :],
                                start=(ic == 0), stop=(ic == IC - 1))
                        yw = ywpool.tile([128, 512], pdt, tag="yw")
                        if sw < 128:
                            nc.gpsimd.memset(yw[sw:128, :], 0.0)
                        nc.vector.tensor_scalar(
                            yw[0:sw, :], py[0:sw, :],
                            w_col[0:sw, 2 * e + sb:2 * e + sb + 1], None,
                            op0=Alu.mult)
                        nidx = sw // 16
                        nc.gpsimd.dma_scatter_add(
                            partials[dc4][:],
                            yw[:].rearrange("p (a f) -> p a f", a=1),
                            idx16[:, 16 * e + 8 * sb:
                                  16 * e + 8 * sb + nidx],
                            sw, sw, 512, elem_step=512, queue_num=0)
                nc.gpsimd.collective_compute(
                    "ReduceScatter", Alu.add, replica_groups=groups,
                    ins=[partials[dc4][:]], outs=[rs_outs[dc4][:]])

        # ---------------- P8b: shared expert down proj (during RS) --------
        # computed fully into SBUF so no DMA in this phase waits on the
        # ReduceScatter; only the final combine below depends on rs_out.
        shr_sb = persist.tile([128, D // 512, TT, 512], f32)
        with tc.tile_pool(name="sdpool", bufs=2) as sdpool:
            for dc4 in range(D // 512):
                off = OFF_SHD + dc4 * SD_COLS
                sd_t = sdpool.tile([128, MC, 512], wdt, tag="wsd")
                nc.sync.dma_start(
                    sd_t[:],
                    wcat[:, off:off + SD_COLS].rearrange(
                        "p (c d) -> p c d", d=512))
                for tb in range(TT):
                    po = psum_y.tile([128, 512], f32, tag="py")
                    for mc in range(MC):
                        nc.tensor.matmul(
                            po[:], HsT[:, mc, tb * 128:(tb + 1) * 128],
                            sd_t[:, mc, :],
                            start=(mc == 0), stop=(mc == MC - 1))
                    nc.vector.tensor_copy(shr_sb[:, dc4, tb, :], po[:])

        # ---------------- P9b: final combine (waits on RS) ----------------
        for dc4 in range(D // 512):
            for tb in range(TT):
                rst = work.tile([128, 512], pdt, tag="rst")
                nc.sync.dma_start(
                    rst[:], rs_outs[dc4][tb * 128:(tb + 1) * 128, :])
                fin = work.tile([128, 512], f32, tag="fin")
                nc.vector.tensor_tensor(
                    fin[:], shr_sb[:, dc4, tb, :], rst[:], op=Alu.add)
                nc.sync.dma_start(
                    out[tb * 128:(tb + 1) * 128,
                        dc4 * 512:(dc4 + 1) * 512], fin[:])

    nc.finalize()
    return nc


_NC_CACHE = {}


def get_nc(gemm_mode=None):
    gemm_mode = gemm_mode or GEMM_MODE
    if gemm_mode not in _NC_CACHE:
        _NC_CACHE[gemm_mode] = _build(gemm_mode)
    return _NC_CACHE[gemm_mode]


def make_in_maps(inputs, gemm_mode=None):
    """Shard + pre-lay-out full inputs into the 8 per-core input maps."""
    import ml_dtypes

    gemm_mode = gemm_mode or GEMM_MODE
    wnp = ml_dtypes.bfloat16 if gemm_mode == "bf16" else np.float32

    x = np.asarray(inputs["hidden_states"], np.float32).reshape(T, D)
    router_w = np.asarray(inputs["router_w"], np.float32)
    e_bias = np.asarray(inputs["e_bias"], np.float32).reshape(E)
    W_gate = np.asarray(inputs["W_gate"], np.float32)
    W_up = np.asarray(inputs["W_up"], np.float32)
    W_down = np.asarray(inputs["W_down"], np.float32)
    shared_gate = np.asarray(inputs["shared_gate"], np.float32)
    shared_up = np.asarray(inputs["shared_up"], np.float32)
    shared_down = np.asarray(inputs["shared_down"], np.float32)

    x_w = np.ascontiguousarray(x).astype(wnp)

    # shared blob pieces (identical on every core)
    shgu = np.empty((128, MC, 2, SG_COLS), np.float32)
    # sg_t[p, dc, j] = shared_gate[mc*128+j, dc*128+p]
    sg4 = shared_gate.reshape(MC, 128, DC, 128)      # [mc, j, dc, p]
    su4 = shared_up.reshape(MC, 128, DC, 128)
    shgu[:, :, 0, :] = np.transpose(sg4, (3, 0, 2, 1)).reshape(128, MC, -1)
    shgu[:, :, 1, :] = np.transpose(su4, (3, 0, 2, 1)).reshape(128, MC, -1)
    shgu = shgu.reshape(128, -1)
    # sd_t[p, mc, d] = shared_down[dc4*512+d, mc*128+p]
    sd4 = shared_down.reshape(4, 512, MC, 128)       # [dc4, d, mc, p]
    shd = np.transpose(sd4, (3, 0, 2, 1)).reshape(128, -1)

    # router small blob (identical on every core except sloc)
    # rwT_sb[p, dc, e] = router_w[e, dc*128+p]
    rw3 = router_w.reshape(E, DC, 128)               # [e, dc, p]
    rwL = np.transpose(rw3, (2, 1, 0)).reshape(128, -1)
    eb = np.broadcast_to(e_bias[None, :], (128, E))

    in_maps = []
    for c in range(NCORES):
        sl = np.zeros((128, 36), np.float32)
        for j in range(EL):
            sl[EL * c + j, j] = 1.0
            sl[E + EL * c + j, 32 + j] = 1.0
        scat = np.concatenate([rwL, eb, sl], axis=1).astype(np.float32)

        # xT_in[p, dc*TL + t] = x[c*TL + t, dc*128 + p]
        xo = x[TL * c:TL * (c + 1)]                  # [TL, D]
        xo3 = xo.reshape(TL, DC, 128)                # [t, dc, p]
        xT_in = np.ascontiguousarray(
            np.transpose(xo3, (2, 1, 0)).reshape(128, -1))

        # expert blob
        eparts = []
        for e in range(EL):
            ge = EL * c + e
            # wg_t[p, dc, i2] for ic: = W_gate[ge][dc*128+p, ic*128+i2]
            wg4 = W_gate[ge].reshape(DC, 128, IC, 128)   # [dc, p, ic, i2]
            wu4 = W_up[ge].reshape(DC, 128, IC, 128)
            gu = np.empty((128, IC, 2, WG_COLS), np.float32)
            gu[:, :, 0, :] = np.transpose(
                wg4, (1, 2, 0, 3)).reshape(128, IC, -1)
            gu[:, :, 1, :] = np.transpose(
                wu4, (1, 2, 0, 3)).reshape(128, IC, -1)
            eparts.append(gu.reshape(128, -1))
            # wd_t[p, ic, d] for dc4: = W_down[ge][ic*128+p, dc4*512+d]
            wd4 = W_down[ge].reshape(IC, 128, 4, 512)    # [ic, p, dc4, d]
            eparts.append(np.transpose(
                wd4, (1, 2, 0, 3)).reshape(128, -1))
        wcat = np.concatenate([shgu, shd] + eparts, axis=1).astype(wnp)
        assert wcat.shape[1] == WCOLS, wcat.shape

        in_maps.append({
            "x_full": x_w,
            "xT_in": xT_in,
            "wcat": np.ascontiguousarray(wcat),
            "scat": np.ascontiguousarray(scat),
        })
    return in_maps


def kernel(**inputs):
    from concourse.bass_utils import run_bass_kernel_spmd

    nc = get_nc()
    in_maps = make_in_maps(inputs)
    res = run_bass_kernel_spmd(
        nc, in_maps, core_ids=list(range(NCORES)), trace=False)
    out = np.concatenate([res.results[c]["out"] for c in range(NCORES)], axis=0)
    return out.reshape(B, S, D)


kernel.last_exec_time_ns = None
